# revision 2
# baseline (speedup 1.0000x reference)
"""Trainium2 Bass kernel for nn_LBLHighwayBiLm — fp32r matmuls, fp16 elementwise.

Model (hardcoded): L=2 layers x {fw,bw} directions. Per layer+direction:
  5-tap windowed sum along seq (learned pad vectors), then a 2-deep AllenNLP
  Highway stack (H=1024 -> 2H proj, sigmoid gate + relu), residual from the
  layer input for l>0. Output: [L, B, S, 2H] = concat(f, bw).

Strategy (data-parallel over batch, 8 cores, B=32 -> BL=4/core). The PE is
the bottleneck (4096 matmul instructions of [128x128]@[128x512] fp32r at
~228ns each ~= 935us); everything else is organized to hide under it:
  - fp16 host I/O + elementwise staging; window sums on DVE as 5
    scalar_tensor_tensor taps (fp16 stage in, fp32r accumulator out, 2x mode).
  - Highway projections on PE as fp32r matmuls accumulating K=1024 in PSUM.
    Weights host-pre-rounded to fp32r (11-bit mantissa RNE).
  - ACT: r = Relu(psum + b), g = Sigmoid(psum + b) -> fp16.
  - Gate combine on DVE stt chain: d=x-r; m=d*g; o=m+r (+res for l>0).
  - Work is split per (jj feature pair, th batch-half): sublayer-1 matmuls
    for th=0 start as soon as th=0's combines finish, while the PE is still
    busy with th=1 — hides the ACT/DVE tails inside the PE stream.
  - Layer-0 outputs stream to DRAM scratch (fp16) and back for layer-1's
    window sum + residual; also DMA'd to the kernel output (fp16, host
    converts to fp32 and reassembles).
"""

import sys

for _p in ("/opt/trn_rl_repo", "/opt/pypackages"):
    if _p not in sys.path:
        sys.path.insert(0, _p)

import numpy as np

import concourse.bass as bass
import concourse.tile as tile
from concourse import mybir, bacc
from concourse import bass_utils

L = 2
NH = 2
WIDTH = 4
H = 1024
B, S = 32, 512
CORES = 8
BL = B // CORES          # 4 batch per core
KT = H // 128            # 8 contraction tiles
NT = 2 * H // 128        # 16 output feature tiles
PSEQ = S + 2 * WIDTH     # padded seq length 520
NTH = 2                  # batch halves
TB = BL // NTH           # batches per half = 2

FP32 = mybir.dt.float32
FP16 = mybir.dt.float16
FP32R = mybir.dt.float32r
AF = mybir.ActivationFunctionType
OP = mybir.AluOpType


def rne_round_fp32(x: np.ndarray, mbits: int = 11) -> np.ndarray:
    """Round fp32 to `mbits` explicit mantissa bits (RNE) — the fp32r format."""
    u = np.ascontiguousarray(x, dtype=np.float32).view(np.uint32).astype(np.uint64)
    shift = 23 - mbits
    bias = ((u >> shift) & 1) + ((1 << (shift - 1)) - 1)
    u = (u + bias) & ~np.uint64((1 << shift) - 1)
    return (u & 0xFFFFFFFF).astype(np.uint32).view(np.float32).reshape(x.shape)


def build_nc(loop_n: int = 1):
    nc = bacc.Bacc("TRN2", target_bir_lowering=False, debug=False)

    x_t = nc.dram_tensor("x_t", [H, BL, S], FP16, kind="ExternalInput").ap()
    fwpad = nc.dram_tensor("fwpad", [L, H, WIDTH], FP16, kind="ExternalInput").ap()
    bwpad = nc.dram_tensor("bwpad", [L, H, WIDTH], FP16, kind="ExternalInput").ap()
    fw_w = nc.dram_tensor("fw_w", [L, WIDTH + 1], FP32, kind="ExternalInput").ap()
    bw_w = nc.dram_tensor("bw_w", [L, WIDTH + 1], FP32, kind="ExternalInput").ap()
    fw_W = nc.dram_tensor("fw_W", [L, NH, H, 2 * H], FP32R, kind="ExternalInput").ap()
    bw_W = nc.dram_tensor("bw_W", [L, NH, H, 2 * H], FP32R, kind="ExternalInput").ap()
    fw_b = nc.dram_tensor("fw_b", [L, NH, 2 * H], FP32, kind="ExternalInput").ap()
    bw_b = nc.dram_tensor("bw_b", [L, NH, 2 * H], FP32, kind="ExternalInput").ap()
    out = nc.dram_tensor("out", [L, 2, H, BL, S], FP16, kind="ExternalOutput").ap()

    args = (x_t, fwpad, bwpad, fw_w, bw_w, fw_W, bw_W, fw_b, bw_b, out)
    with tile.TileContext(nc) as tc:
        if loop_n == 1:
            _emit(tc, nc, *args)
        else:
            with tc.For_i(0, loop_n, 1):
                _emit(tc, nc, *args)
    nc.compile()
    return nc


def _emit(tc, nc, x_t, fwpad, bwpad, fw_w, bw_w, fw_W, bw_W, fw_b, bw_b, out):
    from contextlib import ExitStack
    ctx = ExitStack()
    stg_pool = ctx.enter_context(tc.tile_pool(name="stg", bufs=2))
    ws_pool = ctx.enter_context(tc.tile_pool(name="ws", bufs=2))
    x0_pool = ctx.enter_context(tc.tile_pool(name="x0", bufs=2))
    w_pool = ctx.enter_context(tc.tile_pool(name="wts", bufs=3))
    rg_pool = ctx.enter_context(tc.tile_pool(name="rg", bufs=2))
    ot_pool = ctx.enter_context(tc.tile_pool(name="ot", bufs=2))
    consts = ctx.enter_context(tc.tile_pool(name="consts", bufs=2))
    psum = ctx.enter_context(tc.tile_pool(name="psum", bufs=2, space="PSUM"))
    dram = ctx.enter_context(tc.tile_pool(name="dram", bufs=1, space="DRAM"))

    scr = [dram.tile([H, BL, S], FP16, tag=f"scr{d}", name=f"scr{d}")
           for d in range(2)]

    W_by_dir = (fw_W, bw_W)
    b_by_dir = (fw_b, bw_b)
    w_by_dir = (fw_w, bw_w)

    def load_stage(l, th, src_dram):
        """Padded staging tile [128, KT, TB, PSEQ] fp16 for batch-half th."""
        stg = stg_pool.tile([128, KT, TB, PSEQ], FP16, tag="stg", name="stg")
        for kt in range(KT):
            src = bass.AP(
                tensor=src_dram.tensor,
                offset=src_dram.offset + kt * 128 * BL * S + th * TB * S,
                ap=[[BL * S, 128], [S, TB], [1, S]])
            nc.sync.dma_start(out=stg[:, kt, :, WIDTH:WIDTH + S], in_=src)
            fp = bass.AP(tensor=fwpad.tensor,
                         offset=fwpad.offset + (l * H + kt * 128) * WIDTH,
                         ap=[[WIDTH, 128], [0, TB], [1, WIDTH]])
            nc.sync.dma_start(out=stg[:, kt, :, 0:WIDTH], in_=fp)
            bp = bass.AP(tensor=bwpad.tensor,
                         offset=bwpad.offset + (l * H + kt * 128) * WIDTH,
                         ap=[[WIDTH, 128], [0, TB], [1, WIDTH]])
            nc.sync.dma_start(out=stg[:, kt, :, WIDTH + S:PSEQ], in_=bp)
        return stg

    for l in range(L):
        stgs = {}
        if l == 0:
            for th in range(NTH):
                stgs[th] = load_stage(0, th, x_t)

        for d in range(2):
            if l == 1:
                for th in range(NTH):
                    stgs[th] = load_stage(1, th, scr[d])

            wsrc = w_by_dir[d]
            wt = consts.tile([128, WIDTH + 1], FP32, tag="wt", name="wt")
            nc.sync.dma_start(
                out=wt,
                in_=bass.AP(tensor=wsrc.tensor,
                            offset=wsrc.offset + l * (WIDTH + 1),
                            ap=[[0, 128], [1, WIDTH + 1]]))

            offs = 0 if d == 0 else WIDTH

            # --- window sums into fp32r, per batch-half
            ws_th = {}
            for th in range(NTH):
                ws = ws_pool.tile([128, KT, TB, S], FP32R, tag="ws", name="ws")
                stg = stgs[th]
                for kt in range(KT):
                    nc.vector.tensor_scalar(
                        ws[:, kt], stg[:, kt, :, offs:offs + S], wt[:, 0:1],
                        None, op0=OP.mult)
                    for k in range(1, WIDTH + 1):
                        nc.vector.scalar_tensor_tensor(
                            ws[:, kt], stg[:, kt, :, offs + k:offs + k + S],
                            wt[:, k:k + 1], ws[:, kt], op0=OP.mult, op1=OP.add)
                ws_th[th] = ws

            x0_th = {}
            for th in range(NTH):
                x0_th[th] = x0_pool.tile([128, KT, TB, S], FP32R, tag="x0",
                                         name="x0")
            _sublayer(tc, nc, w_pool, rg_pool, consts, psum,
                      W_by_dir[d], b_by_dir[d], l, 0, ws_th, x0_out=x0_th)
            _sublayer(tc, nc, w_pool, rg_pool, consts, psum,
                      W_by_dir[d], b_by_dir[d], l, 1, x0_th,
                      ot_pool=ot_pool, out_dram=out,
                      scr_dram=scr[d] if l == 0 else None,
                      res_stgs=stgs if l == 1 else None, l_idx=l, d_idx=d)

    ctx.close()


def _sublayer(tc, nc, w_pool, rg_pool, consts, psum, W_src, b_src, l, i,
              x_th, x0_out=None, ot_pool=None, out_dram=None, scr_dram=None,
              res_stgs=None, l_idx=None, d_idx=None):
    bt = consts.tile([128, NT], FP32, tag="bt", name="bt")
    b_ap = b_src[l, i, :]
    nc.sync.dma_start(out=bt, in_=b_ap.rearrange("(n p) -> p n", p=128))

    Wv = W_src[l, i].rearrange("(kt p) c -> p kt c", p=128)

    for jj in range(KT):
        Wnl = w_pool.tile([128, KT, 128], FP32R, tag="Wnl", name="Wnl")
        nc.sync.dma_start(out=Wnl, in_=Wv[:, :, bass.ts(jj, 128)])
        Wsg = w_pool.tile([128, KT, 128], FP32R, tag="Wsg", name="Wsg")
        nc.sync.dma_start(out=Wsg, in_=Wv[:, :, bass.ts(jj + KT, 128)])

        for th in range(NTH):
            x_in = x_th[th]
            pnl = psum.tile([128, TB, S], FP32, tag="pnl", name="pnl")
            psg = psum.tile([128, TB, S], FP32, tag="psg", name="psg")
            for k in range(KT):
                for tb in range(TB):
                    nc.tensor.matmul(pnl[:, tb], Wnl[:, k], x_in[:, k, tb],
                                     start=(k == 0), stop=(k == KT - 1))
            for k in range(KT):
                for tb in range(TB):
                    nc.tensor.matmul(psg[:, tb], Wsg[:, k], x_in[:, k, tb],
                                     start=(k == 0), stop=(k == KT - 1))

            r = rg_pool.tile([128, TB, S], FP16, tag="r", name="r")
            nc.scalar.activation(r, pnl, AF.Relu, bias=bt[:, jj:jj + 1])
            g = rg_pool.tile([128, TB, S], FP16, tag="g", name="g")
            nc.scalar.activation(g, psg, AF.Sigmoid,
                                 bias=bt[:, KT + jj:KT + jj + 1])

            dd = rg_pool.tile([128, TB, S], FP16, tag="dd", name="dd")
            nc.vector.scalar_tensor_tensor(dd, x_in[:, jj], 1.0, r,
                                           op0=OP.mult, op1=OP.subtract)
            nc.vector.scalar_tensor_tensor(dd, dd, 1.0, g,
                                           op0=OP.mult, op1=OP.mult)
            if x0_out is not None:
                nc.vector.scalar_tensor_tensor(x0_out[th][:, jj], dd, 1.0, r,
                                               op0=OP.mult, op1=OP.add)
            else:
                ot = ot_pool.tile([128, TB, S], FP16, tag="ot", name="ot")
                nc.vector.scalar_tensor_tensor(ot, dd, 1.0, r,
                                               op0=OP.mult, op1=OP.add)
                if res_stgs is not None:
                    res = res_stgs[th][:, jj, :, WIDTH:WIDTH + S]
                    nc.vector.scalar_tensor_tensor(ot, ot, 1.0, res,
                                                   op0=OP.mult, op1=OP.add)
                dst = out_dram[l_idx, d_idx, bass.ts(jj, 128),
                               th * TB:(th + 1) * TB, :]
                nc.sync.dma_start(out=dst, in_=ot)
                if scr_dram is not None:
                    sdst = scr_dram[bass.ts(jj, 128), th * TB:(th + 1) * TB, :]
                    nc.sync.dma_start(out=sdst, in_=ot)


def prepare_in_maps(inputs, fw_pad, bw_pad, fw_w, bw_w,
                    fw_hw_W, fw_hw_b, bw_hw_W, bw_hw_b):
    fw_W_r = rne_round_fp32(fw_hw_W)
    bw_W_r = rne_round_fp32(bw_hw_W)
    fwpad_t = np.ascontiguousarray(np.transpose(
        np.asarray(fw_pad, dtype=np.float32), (0, 2, 1))).astype(np.float16)
    bwpad_t = np.ascontiguousarray(np.transpose(
        np.asarray(bw_pad, dtype=np.float32), (0, 2, 1))).astype(np.float16)
    common = {
        "fwpad": fwpad_t, "bwpad": bwpad_t,
        "fw_w": np.ascontiguousarray(fw_w, dtype=np.float32),
        "bw_w": np.ascontiguousarray(bw_w, dtype=np.float32),
        "fw_W": fw_W_r, "bw_W": bw_W_r,
        "fw_b": np.ascontiguousarray(fw_hw_b, dtype=np.float32),
        "bw_b": np.ascontiguousarray(bw_hw_b, dtype=np.float32),
    }
    x = np.asarray(inputs, dtype=np.float32)
    in_maps = []
    for c in range(CORES):
        shard = x[c * BL:(c + 1) * BL]
        x_feat = np.ascontiguousarray(
            np.transpose(shard, (2, 0, 1))).astype(np.float16)  # [H, BL, S]
        in_maps.append({"x_t": x_feat, **common})
    return in_maps


def assemble_output(results):
    full = np.empty((L, B, S, 2 * H), dtype=np.float32)
    for c, r in enumerate(results):
        o = np.asarray(r["out"]).astype(np.float32)  # [L, 2, H, BL, S]
        full[:, c * BL:(c + 1) * BL] = np.transpose(
            o, (0, 3, 4, 1, 2)).reshape(L, BL, S, 2 * H)
    return full


_NC_CACHE = None


def kernel(inputs, masks, fw_pad, bw_pad, fw_w, bw_w,
           fw_hw_W, fw_hw_b, bw_hw_W, bw_hw_b):
    del masks  # all-ones; unused by the reference computation
    global _NC_CACHE
    if _NC_CACHE is None:
        _NC_CACHE = build_nc()
    in_maps = prepare_in_maps(inputs, fw_pad, bw_pad, fw_w, bw_w,
                              fw_hw_W, fw_hw_b, bw_hw_W, bw_hw_b)
    res = bass_utils.run_bass_kernel_spmd(_NC_CACHE, in_maps,
                                          core_ids=list(range(CORES)))
    return assemble_output(res.results)


if __name__ == "__main__":
    nc = build_nc()
    print("built ok")
